# revision 44
# baseline (speedup 1.0000x reference)
"""Trainium2 Bass kernel for nn_AttentionMechanism (location-sensitive additive
attention, B=32 T=1500 E=D=A=512, conv C=10 K=201).

Strategy (8 NeuronCores, data-parallel over batch, 4 batches/core):
  chunk-major over T (3 chunks of 500), batches inner:
    encT = enc[b].T                       (PE transposes of bf16 tiles)
    cfT[c,t] = sum_k conv_w1d[c,k] aw_pad[t+k]   (Hankel matmul, K=201)
    pre.T[a, t] = sum_e W_enc[e,a] enc[t,e] + sum_c W_conv[c,a] cfT[c,t]
                  (+ per-a bias = dec[b] @ W_dec + b_enc, folded into tanh)
    tanhT = tanh(pre.T + bias)            (ScalarE, PSUM -> SBUF bf16)
    energy[t] = sum_a V[a] tanhT[a, t]    (PE, m=1 rows at partition 32b)
  masked softmax over T for all 4 batches at once (partitions {0,32,64,96});
  per-chunk mask-multiply + max run as soon as each chunk's energies land.
  context[b] = sum_t aw[t] enc[b,t,:]     (PE, aw.T tiles as lhsT)

The conv input is the Hankel matrix S[k,t] = aw_pad[t+k], built by an
overlapping-window DMA directly from DRAM (aw_pad is host-padded bf16).

kernel(**inputs) takes the FULL unsharded inputs (names as in
reference.setup_inputs) and returns (context [B,1,E], aw [B,T,1]).
"""

import numpy as np
import ml_dtypes

import concourse.bacc as bacc
import concourse.mybir as mybir
import concourse.tile as tile
from concourse.bass_types import AP
from concourse.bass_utils import run_bass_kernel_spmd

F32 = mybir.dt.float32
BF16 = mybir.dt.bfloat16

B, T, E, D, A, C, KW = 32, 1500, 512, 512, 512, 10, 201
NCORES = 8
NB = B // NCORES              # batches per core
TPAD = 1704                   # aw_pad length (>= T + KW - 1 = 1700)
NCH = 3                       # chunks = 512-row super-tiles of T
PS = [128, 128, 119]          # partitions per super-tile (rows = 4*P)
CW = [512, 512, 476]          # chunk widths (= 4*P)
Q0 = [0, 512, 1024]           # chunk starts (same in t-order and q-order)
KA = A // 128                 # 4
KE = E // 128
KD = D // 128
KW2 = KW - 128                # second Hankel k-tile: 73

# permutation: stored column q = 512*s + r*P + j  <->  time t = 512*s + 4*j + r
def _perm():
    p = np.empty(T, dtype=np.int64)
    for sblk in range(NCH):
        P = PS[sblk]
        for r in range(4):
            p[Q0[sblk] + r * P : Q0[sblk] + (r + 1) * P] = (
                Q0[sblk] + 4 * np.arange(P) + r
            )
    return p

PERM = _perm()

TRACE = False
LAST_EXEC_NS = None
LAST_RESULTS = None


def _build():
    nc = bacc.Bacc(trn_type="TRN2", debug=False, dynamic_dma_scratch_size=16384)

    enc = nc.dram_tensor("enc", [NB, T, E], F32, kind="ExternalInput").ap()
    dec = nc.dram_tensor("dec", [NB, D], F32, kind="ExternalInput").ap()
    awp = nc.dram_tensor("awp", [NB, TPAD], BF16, kind="ExternalInput").ap()
    msk = nc.dram_tensor("msk", [NB, T], F32, kind="ExternalInput").ap()
    wenc = nc.dram_tensor("wenc", [128, KE * A], BF16, kind="ExternalInput").ap()
    wdec = nc.dram_tensor("wdec", [128, KD * A], BF16, kind="ExternalInput").ap()
    cwt = nc.dram_tensor("cwt", [KW, C], BF16, kind="ExternalInput").ap()
    wcv = nc.dram_tensor("wcv", [C, A], BF16, kind="ExternalInput").ap()
    vt = nc.dram_tensor("vt", [128, KA], BF16, kind="ExternalInput").ap()
    benc = nc.dram_tensor("benc", [1, A], BF16, kind="ExternalInput").ap()

    ctxo = nc.dram_tensor("ctxo", [NB, E], F32, kind="ExternalOutput").ap()
    awo = nc.dram_tensor("awo", [NB, T], F32, kind="ExternalOutput").ap()

    with tile.TileContext(nc) as tc:
        with (
            tc.tile_pool(name="sb", bufs=1) as sb,
            tc.tile_pool(name="ps_en", bufs=1, space="PSUM") as ps_en_pool,
        ):
            # --- identity (vector memset so Q7 only runs affine_select) ---
            ident = sb.tile([128, 128], BF16, tag="ident")
            nc.vector.memset(ident, 0.0)
            nc.gpsimd.affine_select(
                out=ident, in_=ident,
                compare_op=mybir.AluOpType.not_equal,
                fill=1.0, base=0, pattern=[[-1, 128]], channel_multiplier=1,
            )

            # --- enc loads (4 rows per partition -> 4KB descriptor runs);
            # encn[p, b, s, r, e] = enc[b, s*512 + 4p + r, e]
            # s in {0,1}: p<128; s=2 holds rows 1024:1500 (476=4*119, p<119).
            encn = [
                sb.tile([128, 3, 4, E], BF16, tag=f"encn{b}", name=f"encn{b}")
                for b in range(NB)
            ]

            # All-SWDGE per-batch loads, interleaved enc/hankel (this
            # pattern empirically gives timely per-batch completion).
            def load_enc_swdge(b):
                nc.gpsimd.dma_start(
                    out=encn[b][:, 0:2, :, :],
                    in_=enc[b, 0:1024, :].rearrange("(s p r) e -> p s r e", p=128, r=4),
                )
                nc.gpsimd.dma_start(
                    out=encn[b][0:119, 2, :, :],
                    in_=enc[b, 1024:1500, :].rearrange("(p r) e -> p r e", r=4),
                )

            # batch 0 in two drained SWDGE segments: bulk (chunks 0-1)
            # flushes first, then tail + hankel 0
            nc.gpsimd.dma_start(
                out=encn[0][:, 0:2, :, :],
                in_=enc[0, 0:1024, :].rearrange("(s p r) e -> p s r e", p=128, r=4),
            )
            nc.gpsimd.drain()
            nc.gpsimd.dma_start(
                out=encn[0][0:119, 2, :, :],
                in_=enc[0, 1024:1500, :].rearrange("(p r) e -> p r e", r=4),
            )

            # --- Hankel windows, per batch, on the ACT ring ---------------
            hank1 = sb.tile([128, NB, T], BF16, tag="hank1")
            hank2 = sb.tile([KW2, NB, T], BF16, tag="hank2")

            def load_hank(b):
                nc.gpsimd.dma_start(
                    out=hank1[:, b, :],
                    in_=AP(awp.tensor, b * TPAD, [[1, 128], [1, T]]),
                )
                nc.gpsimd.dma_start(
                    out=hank2[:, b, :],
                    in_=AP(awp.tensor, b * TPAD + 128, [[1, KW2], [1, T]]),
                )

            load_hank(0)
            # flush the SWDGE queue so batch 0's completion semaphores fire
            # before batches 1-3 are queued behind them
            nc.gpsimd.drain()

            # --- small consts first on the sync ring, then big weights ----
            dec32 = sb.tile([NB, D], F32, tag="dec32")
            nc.sync.dma_start(out=dec32, in_=dec)
            v_sb = sb.tile([128, KA], BF16, tag="v")
            nc.sync.dma_start(out=v_sb, in_=vt)
            cwt1_sb = sb.tile([128, C], BF16, tag="cwt1")
            nc.sync.dma_start(out=cwt1_sb, in_=cwt[0:128, :])
            cwt2_sb = sb.tile([KW2, C], BF16, tag="cwt2")
            nc.sync.dma_start(out=cwt2_sb, in_=cwt[128:KW, :])
            wcv_sb = sb.tile([C, A], BF16, tag="wcv")
            nc.sync.dma_start(out=wcv_sb, in_=wcv)
            benc_sb = sb.tile([1, A], BF16, tag="benc")
            nc.sync.dma_start(out=benc_sb, in_=benc)
            wenc_sb = sb.tile([128, KE, A], BF16, tag="wenc")
            nc.sync.dma_start(out=wenc_sb, in_=wenc.rearrange("p (k a) -> p k a", k=KE))
            wdec_sb = sb.tile([128, KD, A], BF16, tag="wdec")
            nc.sync.dma_start(out=wdec_sb, in_=wdec.rearrange("p (k a) -> p k a", k=KD))


            ones4 = sb.tile([1, NB], BF16, tag="ones4")
            nc.vector.memset(ones4, 1.0)
            msk_sb = sb.tile([128, T], F32, tag="msk")
            nc.vector.memset(msk_sb, 0.0)
            for b in range(NB):
                nc.sync.dma_start(
                    out=msk_sb[32 * b : 32 * b + 1, :], in_=msk[b : b + 1, :]
                )

            # energy accumulators [128, 512] fp32, one bank per chunk;
            # memset so junk partitions read as 0 later.
            ps_en = [
                ps_en_pool.tile(
                    [128, 512], F32, tag=f"en{c}", bufs=1, name=f"ps_en{c}"
                )
                for c in range(NCH)
            ]
            for c in range(NCH):
                nc.vector.memset(ps_en[c], 0.0)

            # ---- dec bias: bias[a, b] = (dec @ W_dec)[b, a] + b_enc[a]
            bias_sb = sb.tile([128, KA, NB], F32, tag="bias")
            with tc.tile_pool(name="ps0", bufs=1, space="PSUM") as ps0:
                decbf = sb.tile([NB, D], BF16, tag="decbf")
                nc.vector.tensor_copy(decbf, dec32)
                ps_dec = ps0.tile([128, KD, NB], BF16, tag="psdec", bufs=1)
                for kd in range(KD):
                    nc.tensor.transpose(
                        ps_dec[:, kd, :],
                        decbf[0:NB, 128 * kd : 128 * (kd + 1)],
                        ident[0:NB, 0:NB],
                    )
                decT = sb.tile([128, KD, NB], BF16, tag="decT")
                nc.vector.tensor_copy(decT, ps_dec)

                for ka in range(KA):
                    ps_b = ps0.tile([128, NB], F32, tag="psbias", bufs=1)
                    for kd in range(KD):
                        nc.tensor.matmul(
                            ps_b,
                            wdec_sb[:, kd, 128 * ka : 128 * (ka + 1)],
                            decT[:, kd, :],
                            start=(kd == 0),
                            stop=False,
                        )
                    nc.tensor.matmul(
                        ps_b,
                        benc_sb[0:1, 128 * ka : 128 * (ka + 1)],
                        ones4[0:1, :],
                        start=False,
                        stop=True,
                    )
                    nc.vector.tensor_copy(bias_sb[:, ka, :], ps_b)

            # ---------------- phase A: batch-major compute ----------------
            energy = sb.tile([128, T], F32, tag="energy")
            pexp = sb.tile([128, T], F32, tag="pexp")
            pexp_bf = sb.tile([128, T], BF16, tag="pexpbf")
            ssum3 = sb.tile([128, NCH], F32, tag="ssum3")
            nc.vector.memset(ssum3, 0.0)
            zbias = sb.tile([128, 1], F32, tag="zbias")
            nc.vector.memset(zbias, 0.0)
            with tc.tile_pool(name="ps1", bufs=1, space="PSUM") as ps1:
                for b in range(NB):
                    if b + 1 < NB:
                        load_enc_swdge(b + 1)
                        load_hank(b + 1)
                    encT = sb.tile([128, KE, T], BF16, tag="encT", bufs=2)
                    for c in range(NCH):
                        q0, W, P = Q0[c], CW[c], PS[c]
                        for r in range(4):
                            ps_tr = ps1.tile([128, KE, 128], BF16, tag="pstr", bufs=2)
                            for ke in range(KE):
                                nc.tensor.transpose(
                                    ps_tr[:, ke, 0:P],
                                    encn[b][0:P, c, r, 128 * ke : 128 * (ke + 1)],
                                    ident[0:P, 0:P],
                                )
                            nc.vector.tensor_copy(
                                encT[:, :, q0 + r * P : q0 + (r + 1) * P],
                                ps_tr[:, :, 0:P],
                            )
                        # conv stage 1: cfT[c10, q] for this (b, chunk);
                        # the rhs AP permutes the t-linear Hankel into q-order
                        ps_cf = ps1.tile([C, 512], F32, tag="pscf", bufs=1)
                        nc.tensor.matmul(
                            ps_cf[:, 0:W],
                            cwt1_sb,
                            hank1[:, b, q0 : q0 + W].rearrange("p (j r) -> p r j", r=4),
                            start=True, stop=False,
                        )
                        nc.tensor.matmul(
                            ps_cf[:, 0:W],
                            cwt2_sb,
                            hank2[:, b, q0 : q0 + W].rearrange("p (j r) -> p r j", r=4),
                            start=False, stop=True,
                        )
                        cfT = sb.tile([C, 512], BF16, tag="cfT", bufs=2)
                        nc.vector.tensor_copy(cfT[:, 0:W], ps_cf[:, 0:W])

                        preT = sb.tile([128, KA, 512], BF16, tag="preT", bufs=2)
                        for ka in range(KA):
                            ps_pre = ps1.tile([128, 512], F32, tag="pspre", bufs=2)
                            for ke in range(KE):
                                nc.tensor.matmul(
                                    ps_pre[:, 0:W],
                                    wenc_sb[:, ke, 128 * ka : 128 * (ka + 1)],
                                    encT[:, ke, q0 : q0 + W],
                                    start=(ke == 0),
                                    stop=False,
                                )
                            nc.tensor.matmul(
                                ps_pre[:, 0:W],
                                wcv_sb[:, 128 * ka : 128 * (ka + 1)],
                                cfT[:, 0:W],
                                start=False,
                                stop=True,
                            )
                            nc.scalar.activation(
                                preT[:, ka, 0:W],
                                ps_pre[:, 0:W],
                                mybir.ActivationFunctionType.Tanh,
                                bias=bias_sb[:, ka, b : b + 1],
                            )
                        for ka in range(KA):
                            nc.tensor.matmul(
                                ps_en[c][32 * b : 32 * b + 1, 0:W],
                                v_sb[:, ka : ka + 1],
                                preT[:, ka, 0:W],
                                start=(ka == 0),
                                stop=(ka == KA - 1),
                                tile_position=(0, 32 * b),
                            )
                        if b == NB - 1:
                            # all batches done with this chunk: mask + max
                            nc.vector.scalar_tensor_tensor(
                                out=energy[:, q0 : q0 + W],
                                in0=ps_en[c][:, 0:W],
                                scalar=1.0,
                                in1=msk_sb[:, q0 : q0 + W],
                                op0=mybir.AluOpType.mult,
                                op1=mybir.AluOpType.mult,
                            )
                            nc.scalar.activation(
                                pexp[:, q0 : q0 + W],
                                energy[:, q0 : q0 + W],
                                mybir.ActivationFunctionType.Exp,
                                bias=zbias[:, 0:1],
                                accum_out=ssum3[:, c : c + 1],
                            )
                            nc.vector.tensor_copy(
                                pexp_bf[:, q0 : q0 + W], pexp[:, q0 : q0 + W]
                            )

            # ------- phase B: normalization only (off the critical path) --
            ssum = sb.tile([128, 1], F32, tag="ssum")
            nc.vector.tensor_reduce(
                ssum, ssum3, axis=mybir.AxisListType.X, op=mybir.AluOpType.add
            )
            rinv = sb.tile([128, 1], F32, tag="rinv")
            nc.vector.reciprocal(rinv, ssum)
            awf = sb.tile([128, T], F32, tag="awf")
            nc.vector.tensor_scalar(
                awf, pexp, scalar1=rinv[:, 0:1], scalar2=None,
                op0=mybir.AluOpType.mult,
            )
            for b in range(NB):
                nc.sync.dma_start(
                    out=awo[b : b + 1, :], in_=awf[32 * b : 32 * b + 1, :]
                )

            # ---------------- phase C: context ----------------------------
            with tc.tile_pool(name="ps2", bufs=1, space="PSUM") as ps2:
                awT = sb.tile([128, 3, 4, NB], BF16, tag="awT")
                for sblk in range(3):
                    P = PS[sblk]
                    for r in range(4):
                        base = Q0[sblk] + r * P
                        ps_awt = ps2.tile([128, 128], BF16, tag="psawt", bufs=2)
                        nc.tensor.transpose(
                            ps_awt[0:P, :], pexp_bf[:, base : base + P], ident
                        )
                        nc.vector.tensor_copy(
                            awT[0:P, sblk, r, :], ps_awt[0:P, 0:128:32]
                        )
                ps_ctx = ps2.tile([128, E], F32, tag="psctx", bufs=1)
                ctx_sb = sb.tile([128, E], F32, tag="ctxsb")
                for b in range(NB):
                    for sblk in range(3):
                        P = PS[sblk]
                        for r in range(4):
                            nc.tensor.matmul(
                                ps_ctx[32 * b : 32 * b + 1, :],
                                awT[0:P, sblk, r, b : b + 1],
                                encn[b][0:P, sblk, r, :],
                                start=(sblk == 0 and r == 0),
                                stop=(sblk == 2 and r == 3),
                                tile_position=(0, 32 * b),
                            )
                    nc.scalar.activation(
                        ctx_sb[32 * b : 32 * b + 1, :],
                        ps_ctx[32 * b : 32 * b + 1, :],
                        mybir.ActivationFunctionType.Copy,
                        scale=rinv[32 * b : 32 * b + 1, 0:1],
                    )
                    nc.sync.dma_start(
                        out=ctxo[b : b + 1, :], in_=ctx_sb[32 * b : 32 * b + 1, :]
                    )

    nc.compile()
    return nc


_NC = None


def _get_nc():
    global _NC
    if _NC is None:
        _NC = _build()
    return _NC


def kernel(enc_out, dec_out, aw_step, W_enc, b_enc, W_dec, W_conv, V, conv_w, x_lens):
    global LAST_EXEC_NS, LAST_RESULTS
    enc_out = np.asarray(enc_out, dtype=np.float32)
    dec_out = np.asarray(dec_out, dtype=np.float32)
    aw_step = np.asarray(aw_step, dtype=np.float32)
    W_enc = np.asarray(W_enc, dtype=np.float32)
    b_enc = np.asarray(b_enc, dtype=np.float32)
    W_dec = np.asarray(W_dec, dtype=np.float32)
    W_conv = np.asarray(W_conv, dtype=np.float32)
    V = np.asarray(V, dtype=np.float32)
    conv_w = np.asarray(conv_w, dtype=np.float32)
    x_lens = np.asarray(x_lens, dtype=np.int32)

    awp = np.zeros((B, TPAD), dtype=np.float32)
    awp[:, 100 : 100 + T] = aw_step[:, :, 0]
    mask = (np.arange(T)[None, :] < x_lens[:, None]).astype(np.float32)
    mask = np.ascontiguousarray(mask[:, PERM])  # device works in q-order
    vt = np.ascontiguousarray(V[:, 0].reshape(KA, 128).T)

    bf = ml_dtypes.bfloat16
    # pre-tile [E, A] -> [128, KE*A]: partition p holds rows {p, 128+p, ...}
    wenc_bf = np.ascontiguousarray(
        W_enc.reshape(KE, 128, A).transpose(1, 0, 2).reshape(128, KE * A)
    ).astype(bf)
    wdec_bf = np.ascontiguousarray(
        W_dec.reshape(KD, 128, A).transpose(1, 0, 2).reshape(128, KD * A)
    ).astype(bf)
    cwt_bf = np.ascontiguousarray(conv_w[:, 0, :].T).astype(bf)   # [KW, C]
    wcv_bf = W_conv.astype(bf)                                    # [C, A]
    vt_bf = vt.astype(bf)
    benc_bf = b_enc.reshape(1, A).astype(bf)
    awp_bf = awp.astype(bf)

    in_maps = []
    for i in range(NCORES):
        s = slice(NB * i, NB * (i + 1))
        in_maps.append(
            {
                "enc": np.ascontiguousarray(enc_out[s]),
                "dec": np.ascontiguousarray(dec_out[s, 0, :]),
                "awp": np.ascontiguousarray(awp_bf[s]),
                "msk": np.ascontiguousarray(mask[s]),
                "wenc": wenc_bf,
                "wdec": wdec_bf,
                "cwt": cwt_bf,
                "wcv": wcv_bf,
                "vt": vt_bf,
                "benc": benc_bf,
            }
        )

    nc = _get_nc()
    res = None
    last_err = None
    for _attempt in range(3):
        try:
            res = run_bass_kernel_spmd(
                nc,
                in_maps,
                list(range(NCORES)),
                trace=TRACE,
                trace_cores=[0] if TRACE else None,
            )
            break
        except Exception as e:  # transient device wedge heals on retry
            last_err = e
    if res is None:
        raise last_err
    LAST_EXEC_NS = res.exec_time_ns
    LAST_RESULTS = res

    context = np.concatenate([r["ctxo"] for r in res.results], axis=0)
    aw_q = np.concatenate([r["awo"] for r in res.results], axis=0)
    aw = np.empty_like(aw_q)
    aw[:, PERM] = aw_q  # back to t-order
    return context.reshape(B, 1, E), aw.reshape(B, T, 1)


# revision 45
# speedup vs baseline: 1.0399x; 1.0399x over previous
"""Trainium2 Bass kernel for nn_AttentionMechanism (location-sensitive additive
attention, B=32 T=1500 E=D=A=512, conv C=10 K=201).

Strategy (8 NeuronCores, data-parallel over batch, 4 batches/core):
  chunk-major over T (3 chunks of 500), batches inner:
    encT = enc[b].T                       (PE transposes of bf16 tiles)
    cfT[c,t] = sum_k conv_w1d[c,k] aw_pad[t+k]   (Hankel matmul, K=201)
    pre.T[a, t] = sum_e W_enc[e,a] enc[t,e] + sum_c W_conv[c,a] cfT[c,t]
                  (+ per-a bias = dec[b] @ W_dec + b_enc, folded into tanh)
    tanhT = tanh(pre.T + bias)            (ScalarE, PSUM -> SBUF bf16)
    energy[t] = sum_a V[a] tanhT[a, t]    (PE, m=1 rows at partition 32b)
  masked softmax over T for all 4 batches at once (partitions {0,32,64,96});
  per-chunk mask-multiply + max run as soon as each chunk's energies land.
  context[b] = sum_t aw[t] enc[b,t,:]     (PE, aw.T tiles as lhsT)

The conv input is the Hankel matrix S[k,t] = aw_pad[t+k], built by an
overlapping-window DMA directly from DRAM (aw_pad is host-padded bf16).

kernel(**inputs) takes the FULL unsharded inputs (names as in
reference.setup_inputs) and returns (context [B,1,E], aw [B,T,1]).
"""

import numpy as np
import ml_dtypes

import concourse.bacc as bacc
import concourse.mybir as mybir
import concourse.tile as tile
from concourse.bass_types import AP
from concourse.bass_utils import run_bass_kernel_spmd

F32 = mybir.dt.float32
BF16 = mybir.dt.bfloat16

B, T, E, D, A, C, KW = 32, 1500, 512, 512, 512, 10, 201
NCORES = 8
NB = B // NCORES              # batches per core
TPAD = 1704                   # aw_pad length (>= T + KW - 1 = 1700)
NCH = 3                       # chunks = 512-row super-tiles of T
PS = [128, 128, 119]          # partitions per super-tile (rows = 4*P)
CW = [512, 512, 476]          # chunk widths (= 4*P)
Q0 = [0, 512, 1024]           # chunk starts (same in t-order and q-order)
KA = A // 128                 # 4
KE = E // 128
KD = D // 128
KW2 = KW - 128                # second Hankel k-tile: 73

# permutation: stored column q = 512*s + r*P + j  <->  time t = 512*s + 4*j + r
def _perm():
    p = np.empty(T, dtype=np.int64)
    for sblk in range(NCH):
        P = PS[sblk]
        for r in range(4):
            p[Q0[sblk] + r * P : Q0[sblk] + (r + 1) * P] = (
                Q0[sblk] + 4 * np.arange(P) + r
            )
    return p

PERM = _perm()

TRACE = False
LAST_EXEC_NS = None
LAST_RESULTS = None


def _build():
    nc = bacc.Bacc(trn_type="TRN2", debug=False, dynamic_dma_scratch_size=16384)

    enc = nc.dram_tensor("enc", [NB, T, E], F32, kind="ExternalInput").ap()
    dec = nc.dram_tensor("dec", [NB, D], F32, kind="ExternalInput").ap()
    awp = nc.dram_tensor("awp", [NB, TPAD], BF16, kind="ExternalInput").ap()
    msk = nc.dram_tensor("msk", [NB, T], F32, kind="ExternalInput").ap()
    wenc = nc.dram_tensor("wenc", [128, KE * A], BF16, kind="ExternalInput").ap()
    wdec = nc.dram_tensor("wdec", [128, KD * A], BF16, kind="ExternalInput").ap()
    cwt = nc.dram_tensor("cwt", [KW, C], BF16, kind="ExternalInput").ap()
    wcv = nc.dram_tensor("wcv", [C, A], BF16, kind="ExternalInput").ap()
    vt = nc.dram_tensor("vt", [128, KA], BF16, kind="ExternalInput").ap()
    benc = nc.dram_tensor("benc", [1, A], BF16, kind="ExternalInput").ap()

    ctxo = nc.dram_tensor("ctxo", [NB, E], F32, kind="ExternalOutput").ap()
    awo = nc.dram_tensor("awo", [NB, T], F32, kind="ExternalOutput").ap()

    with tile.TileContext(nc) as tc:
        with (
            tc.tile_pool(name="sb", bufs=1) as sb,
            tc.tile_pool(name="ps_en", bufs=1, space="PSUM") as ps_en_pool,
        ):
            # --- identity (vector memset so Q7 only runs affine_select) ---
            ident = sb.tile([128, 128], BF16, tag="ident")
            nc.vector.memset(ident, 0.0)
            nc.gpsimd.affine_select(
                out=ident, in_=ident,
                compare_op=mybir.AluOpType.not_equal,
                fill=1.0, base=0, pattern=[[-1, 128]], channel_multiplier=1,
            )

            # --- enc loads (4 rows per partition -> 4KB descriptor runs);
            # encn[p, b, s, r, e] = enc[b, s*512 + 4p + r, e]
            # s in {0,1}: p<128; s=2 holds rows 1024:1500 (476=4*119, p<119).
            encn = [
                sb.tile([128, 3, 4, E], BF16, tag=f"encn{b}", name=f"encn{b}")
                for b in range(NB)
            ]

            # All-SWDGE per-batch loads, interleaved enc/hankel (this
            # pattern empirically gives timely per-batch completion).
            def load_enc_swdge(b):
                nc.gpsimd.dma_start(
                    out=encn[b][:, 0:2, :, :],
                    in_=enc[b, 0:1024, :].rearrange("(s p r) e -> p s r e", p=128, r=4),
                )
                nc.gpsimd.dma_start(
                    out=encn[b][0:119, 2, :, :],
                    in_=enc[b, 1024:1500, :].rearrange("(p r) e -> p r e", r=4),
                )

            load_enc_swdge(0)

            # --- Hankel windows, per batch, on the ACT ring ---------------
            hank1 = sb.tile([128, NB, T], BF16, tag="hank1")
            hank2 = sb.tile([KW2, NB, T], BF16, tag="hank2")

            def load_hank(b):
                nc.gpsimd.dma_start(
                    out=hank1[:, b, :],
                    in_=AP(awp.tensor, b * TPAD, [[1, 128], [1, T]]),
                )
                nc.gpsimd.dma_start(
                    out=hank2[:, b, :],
                    in_=AP(awp.tensor, b * TPAD + 128, [[1, KW2], [1, T]]),
                )

            load_hank(0)
            # flush the SWDGE queue so batch 0's completion semaphores fire
            # before batches 1-3 are queued behind them
            nc.gpsimd.drain()

            # --- small consts first on the sync ring, then big weights ----
            dec32 = sb.tile([NB, D], F32, tag="dec32")
            nc.sync.dma_start(out=dec32, in_=dec)
            v_sb = sb.tile([128, KA], BF16, tag="v")
            nc.sync.dma_start(out=v_sb, in_=vt)
            cwt1_sb = sb.tile([128, C], BF16, tag="cwt1")
            nc.sync.dma_start(out=cwt1_sb, in_=cwt[0:128, :])
            cwt2_sb = sb.tile([KW2, C], BF16, tag="cwt2")
            nc.sync.dma_start(out=cwt2_sb, in_=cwt[128:KW, :])
            wcv_sb = sb.tile([C, A], BF16, tag="wcv")
            nc.sync.dma_start(out=wcv_sb, in_=wcv)
            benc_sb = sb.tile([1, A], BF16, tag="benc")
            nc.sync.dma_start(out=benc_sb, in_=benc)
            wenc_sb = sb.tile([128, KE, A], BF16, tag="wenc")
            nc.sync.dma_start(out=wenc_sb, in_=wenc.rearrange("p (k a) -> p k a", k=KE))
            wdec_sb = sb.tile([128, KD, A], BF16, tag="wdec")
            nc.sync.dma_start(out=wdec_sb, in_=wdec.rearrange("p (k a) -> p k a", k=KD))


            ones4 = sb.tile([1, NB], BF16, tag="ones4")
            nc.vector.memset(ones4, 1.0)
            msk_sb = sb.tile([128, T], F32, tag="msk")
            nc.vector.memset(msk_sb, 0.0)
            for b in range(NB):
                nc.sync.dma_start(
                    out=msk_sb[32 * b : 32 * b + 1, :], in_=msk[b : b + 1, :]
                )

            # energy accumulators [128, 512] fp32, one bank per chunk;
            # memset so junk partitions read as 0 later.
            ps_en = [
                ps_en_pool.tile(
                    [128, 512], F32, tag=f"en{c}", bufs=1, name=f"ps_en{c}"
                )
                for c in range(NCH)
            ]
            for c in range(NCH):
                nc.vector.memset(ps_en[c], 0.0)

            # ---- dec bias: bias[a, b] = (dec @ W_dec)[b, a] + b_enc[a]
            bias_sb = sb.tile([128, KA, NB], F32, tag="bias")
            with tc.tile_pool(name="ps0", bufs=1, space="PSUM") as ps0:
                decbf = sb.tile([NB, D], BF16, tag="decbf")
                nc.vector.tensor_copy(decbf, dec32)
                ps_dec = ps0.tile([128, KD, NB], BF16, tag="psdec", bufs=1)
                for kd in range(KD):
                    nc.tensor.transpose(
                        ps_dec[:, kd, :],
                        decbf[0:NB, 128 * kd : 128 * (kd + 1)],
                        ident[0:NB, 0:NB],
                    )
                decT = sb.tile([128, KD, NB], BF16, tag="decT")
                nc.vector.tensor_copy(decT, ps_dec)

                for ka in range(KA):
                    ps_b = ps0.tile([128, NB], F32, tag="psbias", bufs=1)
                    for kd in range(KD):
                        nc.tensor.matmul(
                            ps_b,
                            wdec_sb[:, kd, 128 * ka : 128 * (ka + 1)],
                            decT[:, kd, :],
                            start=(kd == 0),
                            stop=False,
                        )
                    nc.tensor.matmul(
                        ps_b,
                        benc_sb[0:1, 128 * ka : 128 * (ka + 1)],
                        ones4[0:1, :],
                        start=False,
                        stop=True,
                    )
                    nc.vector.tensor_copy(bias_sb[:, ka, :], ps_b)

            # ---------------- phase A: batch-major compute ----------------
            energy = sb.tile([128, T], F32, tag="energy")
            pexp = sb.tile([128, T], F32, tag="pexp")
            pexp_bf = sb.tile([128, T], BF16, tag="pexpbf")
            ssum3 = sb.tile([128, NCH], F32, tag="ssum3")
            nc.vector.memset(ssum3, 0.0)
            zbias = sb.tile([128, 1], F32, tag="zbias")
            nc.vector.memset(zbias, 0.0)
            with tc.tile_pool(name="ps1", bufs=1, space="PSUM") as ps1:
                for b in range(NB):
                    if b + 1 < NB:
                        load_enc_swdge(b + 1)
                        load_hank(b + 1)
                    encT = sb.tile([128, KE, T], BF16, tag="encT", bufs=2)
                    for c in range(NCH):
                        q0, W, P = Q0[c], CW[c], PS[c]
                        for r in range(4):
                            ps_tr = ps1.tile([128, KE, 128], BF16, tag="pstr", bufs=2)
                            for ke in range(KE):
                                nc.tensor.transpose(
                                    ps_tr[:, ke, 0:P],
                                    encn[b][0:P, c, r, 128 * ke : 128 * (ke + 1)],
                                    ident[0:P, 0:P],
                                )
                            nc.vector.tensor_copy(
                                encT[:, :, q0 + r * P : q0 + (r + 1) * P],
                                ps_tr[:, :, 0:P],
                            )
                        # conv stage 1: cfT[c10, q] for this (b, chunk);
                        # the rhs AP permutes the t-linear Hankel into q-order
                        ps_cf = ps1.tile([C, 512], F32, tag="pscf", bufs=1)
                        nc.tensor.matmul(
                            ps_cf[:, 0:W],
                            cwt1_sb,
                            hank1[:, b, q0 : q0 + W].rearrange("p (j r) -> p r j", r=4),
                            start=True, stop=False,
                        )
                        nc.tensor.matmul(
                            ps_cf[:, 0:W],
                            cwt2_sb,
                            hank2[:, b, q0 : q0 + W].rearrange("p (j r) -> p r j", r=4),
                            start=False, stop=True,
                        )
                        cfT = sb.tile([C, 512], BF16, tag="cfT", bufs=2)
                        nc.vector.tensor_copy(cfT[:, 0:W], ps_cf[:, 0:W])

                        preT = sb.tile([128, KA, 512], BF16, tag="preT", bufs=2)
                        for ka in range(KA):
                            ps_pre = ps1.tile([128, 512], F32, tag="pspre", bufs=2)
                            for ke in range(KE):
                                nc.tensor.matmul(
                                    ps_pre[:, 0:W],
                                    wenc_sb[:, ke, 128 * ka : 128 * (ka + 1)],
                                    encT[:, ke, q0 : q0 + W],
                                    start=(ke == 0),
                                    stop=False,
                                )
                            nc.tensor.matmul(
                                ps_pre[:, 0:W],
                                wcv_sb[:, 128 * ka : 128 * (ka + 1)],
                                cfT[:, 0:W],
                                start=False,
                                stop=True,
                            )
                            nc.scalar.activation(
                                preT[:, ka, 0:W],
                                ps_pre[:, 0:W],
                                mybir.ActivationFunctionType.Tanh,
                                bias=bias_sb[:, ka, b : b + 1],
                            )
                        for ka in range(KA):
                            nc.tensor.matmul(
                                ps_en[c][32 * b : 32 * b + 1, 0:W],
                                v_sb[:, ka : ka + 1],
                                preT[:, ka, 0:W],
                                start=(ka == 0),
                                stop=(ka == KA - 1),
                                tile_position=(0, 32 * b),
                            )
                        if b == NB - 1:
                            # all batches done with this chunk: mask + max
                            nc.vector.scalar_tensor_tensor(
                                out=energy[:, q0 : q0 + W],
                                in0=ps_en[c][:, 0:W],
                                scalar=1.0,
                                in1=msk_sb[:, q0 : q0 + W],
                                op0=mybir.AluOpType.mult,
                                op1=mybir.AluOpType.mult,
                            )
                            nc.scalar.activation(
                                pexp[:, q0 : q0 + W],
                                energy[:, q0 : q0 + W],
                                mybir.ActivationFunctionType.Exp,
                                bias=zbias[:, 0:1],
                                accum_out=ssum3[:, c : c + 1],
                            )
                            nc.vector.tensor_copy(
                                pexp_bf[:, q0 : q0 + W], pexp[:, q0 : q0 + W]
                            )

            # ------- phase B: normalization only (off the critical path) --
            ssum = sb.tile([128, 1], F32, tag="ssum")
            nc.vector.tensor_reduce(
                ssum, ssum3, axis=mybir.AxisListType.X, op=mybir.AluOpType.add
            )
            rinv = sb.tile([128, 1], F32, tag="rinv")
            nc.vector.reciprocal(rinv, ssum)
            awf = sb.tile([128, T], F32, tag="awf")
            nc.vector.tensor_scalar(
                awf, pexp, scalar1=rinv[:, 0:1], scalar2=None,
                op0=mybir.AluOpType.mult,
            )
            for b in range(NB):
                nc.sync.dma_start(
                    out=awo[b : b + 1, :], in_=awf[32 * b : 32 * b + 1, :]
                )

            # ---------------- phase C: context ----------------------------
            with tc.tile_pool(name="ps2", bufs=1, space="PSUM") as ps2:
                awT = sb.tile([128, 3, 4, NB], BF16, tag="awT")
                for sblk in range(3):
                    P = PS[sblk]
                    for r in range(4):
                        base = Q0[sblk] + r * P
                        ps_awt = ps2.tile([128, 128], BF16, tag="psawt", bufs=2)
                        nc.tensor.transpose(
                            ps_awt[0:P, :], pexp_bf[:, base : base + P], ident
                        )
                        nc.vector.tensor_copy(
                            awT[0:P, sblk, r, :], ps_awt[0:P, 0:128:32]
                        )
                ps_ctx = ps2.tile([128, E], F32, tag="psctx", bufs=1)
                ctx_sb = sb.tile([128, E], F32, tag="ctxsb")
                for b in range(NB):
                    for sblk in range(3):
                        P = PS[sblk]
                        for r in range(4):
                            nc.tensor.matmul(
                                ps_ctx[32 * b : 32 * b + 1, :],
                                awT[0:P, sblk, r, b : b + 1],
                                encn[b][0:P, sblk, r, :],
                                start=(sblk == 0 and r == 0),
                                stop=(sblk == 2 and r == 3),
                                tile_position=(0, 32 * b),
                            )
                    nc.scalar.activation(
                        ctx_sb[32 * b : 32 * b + 1, :],
                        ps_ctx[32 * b : 32 * b + 1, :],
                        mybir.ActivationFunctionType.Copy,
                        scale=rinv[32 * b : 32 * b + 1, 0:1],
                    )
                    nc.sync.dma_start(
                        out=ctxo[b : b + 1, :], in_=ctx_sb[32 * b : 32 * b + 1, :]
                    )

    nc.compile()
    return nc


_NC = None


def _get_nc():
    global _NC
    if _NC is None:
        _NC = _build()
    return _NC


def kernel(enc_out, dec_out, aw_step, W_enc, b_enc, W_dec, W_conv, V, conv_w, x_lens):
    global LAST_EXEC_NS, LAST_RESULTS
    enc_out = np.asarray(enc_out, dtype=np.float32)
    dec_out = np.asarray(dec_out, dtype=np.float32)
    aw_step = np.asarray(aw_step, dtype=np.float32)
    W_enc = np.asarray(W_enc, dtype=np.float32)
    b_enc = np.asarray(b_enc, dtype=np.float32)
    W_dec = np.asarray(W_dec, dtype=np.float32)
    W_conv = np.asarray(W_conv, dtype=np.float32)
    V = np.asarray(V, dtype=np.float32)
    conv_w = np.asarray(conv_w, dtype=np.float32)
    x_lens = np.asarray(x_lens, dtype=np.int32)

    awp = np.zeros((B, TPAD), dtype=np.float32)
    awp[:, 100 : 100 + T] = aw_step[:, :, 0]
    mask = (np.arange(T)[None, :] < x_lens[:, None]).astype(np.float32)
    mask = np.ascontiguousarray(mask[:, PERM])  # device works in q-order
    vt = np.ascontiguousarray(V[:, 0].reshape(KA, 128).T)

    bf = ml_dtypes.bfloat16
    # pre-tile [E, A] -> [128, KE*A]: partition p holds rows {p, 128+p, ...}
    wenc_bf = np.ascontiguousarray(
        W_enc.reshape(KE, 128, A).transpose(1, 0, 2).reshape(128, KE * A)
    ).astype(bf)
    wdec_bf = np.ascontiguousarray(
        W_dec.reshape(KD, 128, A).transpose(1, 0, 2).reshape(128, KD * A)
    ).astype(bf)
    cwt_bf = np.ascontiguousarray(conv_w[:, 0, :].T).astype(bf)   # [KW, C]
    wcv_bf = W_conv.astype(bf)                                    # [C, A]
    vt_bf = vt.astype(bf)
    benc_bf = b_enc.reshape(1, A).astype(bf)
    awp_bf = awp.astype(bf)

    in_maps = []
    for i in range(NCORES):
        s = slice(NB * i, NB * (i + 1))
        in_maps.append(
            {
                "enc": np.ascontiguousarray(enc_out[s]),
                "dec": np.ascontiguousarray(dec_out[s, 0, :]),
                "awp": np.ascontiguousarray(awp_bf[s]),
                "msk": np.ascontiguousarray(mask[s]),
                "wenc": wenc_bf,
                "wdec": wdec_bf,
                "cwt": cwt_bf,
                "wcv": wcv_bf,
                "vt": vt_bf,
                "benc": benc_bf,
            }
        )

    nc = _get_nc()
    res = None
    last_err = None
    for _attempt in range(3):
        try:
            res = run_bass_kernel_spmd(
                nc,
                in_maps,
                list(range(NCORES)),
                trace=TRACE,
                trace_cores=[0] if TRACE else None,
            )
            break
        except Exception as e:  # transient device wedge heals on retry
            last_err = e
    if res is None:
        raise last_err
    LAST_EXEC_NS = res.exec_time_ns
    LAST_RESULTS = res

    context = np.concatenate([r["ctxo"] for r in res.results], axis=0)
    aw_q = np.concatenate([r["awo"] for r in res.results], axis=0)
    aw = np.empty_like(aw_q)
    aw[:, PERM] = aw_q  # back to t-order
    return context.reshape(B, 1, E), aw.reshape(B, T, 1)
